# revision 25
# baseline (speedup 1.0000x reference)
"""Trainium2 Bass kernel for DecomposingAttnProcessor (pooled component softmax
cross-attention), sharded over 8 NeuronCores along the latent-token axis S.

Math (per batch-component bc = c*B + b):
    q = x @ Wq ; k = enc @ Wk ; v = enc @ Wv           (per-head, dh = 64)
    scores = (q k^T) * dh^-0.5                          [H, S, E]
    pooled = mean_E scores ; wp = softmax_c(pooled)
    w = softmax_E(scores) * wp
    out = (w v) @ Wo + bo + x

Sharding: each core owns a 512-row slice of S for ALL batch-components; the
component softmax couples only the c axis, which stays on-core.

v2 dataflow (512-row tiles, KV resident in SBUF):
    encoder phase: kT[(bc,j)] = Wk^T encT with fused ksum/E col (projected mean
      key for the pooled path), v0/v1 in natural [e, 64h+d] layout with the E1
      rows replicated to all four 32-row blocks (feeds row-tiled AV matmuls).
    per (b, c):  A: xT via PE transpose; qT = Wq^T xT (bf16, unscaled)
                 B: scoresT per head ([128,512] E0 + 4-head-packed E1 bank),
                    exp with scale=dh^-0.5 fused; AV; den via one-hot matmuls
                    accumulated into a [16,512] PSUM slice at rows 32c
                 C: coef = softmax_c(exp(scale*pooled)) / den  (bf16 stats)
                 D: coef broadcast over dh via DRAM bounce; ao *= coef
                 E: out = aoT^T @ Wo + bo + x
    D/E of (b,c) are emitted one component later so the PE stream stays dense
    while the coef chain resolves.
"""

import sys
from contextlib import ExitStack

sys.path.insert(0, "/opt/trn_rl_repo")

import numpy as np

import concourse.bass as bass  # noqa: E402
from concourse import bacc, mybir  # noqa: E402
from concourse.bass_utils import run_bass_kernel_spmd  # noqa: E402
from concourse.masks import make_identity  # noqa: E402
from concourse.tile import TileContext  # noqa: E402

# Problem dims (hardcoded per spec)
BC, S, D, E, H, C = 8, 4096, 1024, 160, 16, 4
B = BC // C  # 2
DH = D // H  # 64
SCALE = DH**-0.5  # 0.125
N_CORES = 8
S_LOC = S // N_CORES  # 512 rows of S per core
E0, E1 = 128, E - 128  # encoder-token chunks (128 + 32)
ND = D // 128  # 8 chunks of the hidden dim

F32 = mybir.dt.float32
BF16 = mybir.dt.bfloat16
AF = mybir.ActivationFunctionType


def build_body(ctx, tc, d, s_loc):
    nc = tc.nc
    P = 128
    SL = s_loc  # 512

    pools = {}

    def pool(name, bufs, space="SBUF"):
        if name not in pools:
            pools[name] = ctx.enter_context(tc.tile_pool(name=name, bufs=bufs, space=space))
        return pools[name]

    const = pool("const", 1)
    wmat = pool("wmat", 1)  # Wk|Wv bf16 pairs, later reused for Wq
    wop = pool("wo", 1)
    ktp = pool("kt", 1)  # resident kT tiles, all 8 bc
    vp = pool("v", 1)  # resident v0 / v1_4 tiles, all 8 bc
    ksbp = pool("ksb", 1)  # resident block-diag ksum tiles
    xin_p = pool("xin", 1)
    xt_p = pool("xt", 1)
    qt_p = pool("qt", 2)
    w_p = pool("w", 3)  # exp(scores) E0 per-head tiles
    wb_p = pool("wb4", 4)  # exp(scores) E1 4-head-packed tiles
    ao_p = pool("ao", 1)
    stats = pool("stats", 1)
    cb_p = pool("cb", 4)
    xr_p = pool("xr", 2)
    dram = pool("dram", 1, space="DRAM")

    # PSUM pools: 8 banks total
    pmm = pool("pmm", 2, space="PSUM")  # qproj / vproj / O-proj [128,512] f32
    psa = pool("psa", 2, space="PSUM")  # E0 scores [128,512] f32
    pm2 = pool("pm2", 2, space="PSUM")  # transposes / E1-pack / AV
    ppool = pool("ppool", 1, space="PSUM")  # pooled collector, rows 32c+h
    pden = pool("pden", 1, space="PSUM")  # denominator collector, rows 32c+h

    # ---- constants ----
    ident = const.tile([P, P], BF16, tag="ident")
    make_identity(nc, ident)
    ones_row = const.tile([1, P], BF16, tag="ones_row")
    nc.vector.memset(ones_row, 1.0)
    bo_bf = const.tile([1, D], BF16, tag="bo_bf")
    nc.gpsimd.dma_start(out=bo_bf, in_=d["bo"])  # f32 -> bf16 cast DMA

    # one-hot selector columns for the den matmuls:
    #   oneh[h]: [128, 16] with col h = 1 (E0 full-column sum for head h)
    #   ones4[p]: [128, 16] with rows 32g:32g+32 of col 4p+g = 1 (E1 pack p)
    oneh = []
    for h in range(H):
        t = const.tile([P, H], BF16, tag=f"oneh{h}", name=f"oneh{h}")
        nc.vector.memset(t, 0.0)
        nc.vector.memset(t[:, h : h + 1], 1.0)
        oneh.append(t)
    ones4 = []
    for p in range(4):
        t = const.tile([P, H], BF16, tag=f"ones4_{p}", name=f"ones4_{p}")
        nc.vector.memset(t, 0.0)
        for g in range(4):
            nc.vector.memset(t[32 * g : 32 * (g + 1), 4 * p + g : 4 * p + g + 1], 1.0)
        ones4.append(t)

    # ---- load Wk/Wv as bf16 pairs (Wk halves first so the kT chain starts
    # as soon as possible; Wo/bo are loaded after the encoder is emitted) ----
    wkv = []
    for i in range(ND):
        t = wmat.tile([P, 2 * D], BF16, tag=f"w{i}", name=f"wkv{i}")
        nc.gpsimd.dma_start(out=t[:, 0:D], in_=d["Wk"][128 * i : 128 * (i + 1), :])
        wkv.append(t)
    for i in range(ND):
        nc.gpsimd.dma_start(out=wkv[i][:, D : 2 * D], in_=d["Wv"][128 * i : 128 * (i + 1), :])

    # ---- resident KV tiles ----
    kt = {}  # (bc, j) -> [128, E+1] bf16 (col E = projected ksum/E)
    v0 = {}  # bc -> [128, 1024] bf16, rows = E0 tokens, cols = 64h + d
    v14 = {}  # bc -> [128, 1024] bf16, E1 tokens replicated at rows 32g
    ksb = {}  # (bc, j) -> [128, 16] view into ksb_all
    for bc in range(BC):
        for j in range(ND):
            kt[(bc, j)] = ktp.tile([P, E + 1], BF16, tag=f"kt{bc}_{j}", name=f"kt{bc}_{j}")
        v0[bc] = vp.tile([P, H * DH], BF16, tag=f"v0_{bc}", name=f"v0_{bc}")
        v14[bc] = vp.tile([P, H * DH], BF16, tag=f"v14_{bc}", name=f"v14_{bc}")

    # ---- encoder phase: per bc, compute kT (+ksum/E col) and v ----
    with tc.tile_pool(name="enc", bufs=2) as encp:
        for bc in range(BC):
            et0 = encp.tile([P, D], BF16, tag="et0")
            et1 = encp.tile([E1, D], BF16, tag="et1")
            nc.gpsimd.dma_start(out=et0, in_=d["enc"][bc, 0:E0, :])
            nc.gpsimd.dma_start(out=et1, in_=d["enc"][bc, E0:E, :])

            # encT, all 8 chunks in one [128, 8*(E+1)] tile; col 160 of each
            # chunk = esum/E (batched reduce + strided scatter-mul)
            enct = encp.tile([P, ND * (E + 1)], BF16, tag="enct", bufs=1)
            for i in range(ND):
                ps = pm2.tile([P, E], BF16, tag="pm")
                sl = slice(128 * i, 128 * (i + 1))
                nc.tensor.transpose(ps[:, 0:E0], et0[:, sl], ident)
                nc.tensor.transpose(ps[:, E0:E], et1[:, sl], ident[0:E1, 0:E1])
                nc.scalar.activation(enct[:, (E + 1) * i : (E + 1) * i + E], ps, AF.Copy)
            esum = encp.tile([P, ND], F32, tag="esum", bufs=1)
            e3 = enct.rearrange("p (i e) -> p i e", e=E + 1)
            nc.vector.tensor_reduce(esum, e3[:, :, 0:E], axis=mybir.AxisListType.X, op=mybir.AluOpType.add)
            nc.scalar.mul(e3[:, :, E : E + 1], esum, 1.0 / E)

            # kT projection: [dout-chunk, E+1] (col E = projected ksum/E)
            ksb_all = ksbp.tile([P, ND * H], BF16, tag=f"ksb{bc}", name=f"ksb{bc}")
            nc.vector.memset(ksb_all, 0.0)
            for j in range(ND):
                ps = psa.tile([P, E + 1], F32, tag="ps")
                for i in range(ND):
                    nc.tensor.matmul(
                        ps,
                        lhsT=wkv[i][:, 128 * j : 128 * (j + 1)],
                        rhs=enct[:, (E + 1) * i : (E + 1) * (i + 1)],
                        start=(i == 0),
                        stop=(i == ND - 1),
                    )
                t = kt[(bc, j)]
                nc.scalar.activation(t, ps, AF.Copy)
                ksb[(bc, j)] = ksb_all[:, H * j : H * (j + 1)]
                nc.vector.tensor_copy(ksb_all[0:64, H * j + 2 * j : H * j + 2 * j + 1], t[0:64, E : E + 1])
                nc.vector.tensor_copy(ksb_all[64:128, H * j + 2 * j + 1 : H * j + 2 * j + 2], t[64:128, E : E + 1])

            # v projection, natural [token, 64h+d] layout
            for half in range(2):
                cs = slice(D + 512 * half, D + 512 * (half + 1))
                ps = pmm.tile([P, 512], F32, tag="ps")
                for i in range(ND):
                    nc.tensor.matmul(
                        ps,
                        lhsT=enct[:, (E + 1) * i : (E + 1) * i + E0],
                        rhs=wkv[i][:, cs],
                        start=(i == 0),
                        stop=(i == ND - 1),
                    )
                nc.scalar.activation(v0[bc][:, 512 * half : 512 * (half + 1)], ps, AF.Copy)
                ps1 = psa.tile([E1, 512], F32, tag="ps")
                for i in range(ND):
                    nc.tensor.matmul(
                        ps1,
                        lhsT=enct[:, (E + 1) * i + E0 : (E + 1) * i + E],
                        rhs=wkv[i][:, cs],
                        start=(i == 0),
                        stop=(i == ND - 1),
                    )
                nc.scalar.activation(v14[bc][0:E1, 512 * half : 512 * (half + 1)], ps1, AF.Copy)
            for g in range(1, 4):
                nc.vector.tensor_copy(v14[bc][32 * g : 32 * g + E1, :], v14[bc][0:E1, :])

    # ---- load Wq (bf16) into the Wk/Wv slots; Wo/bo now too ----
    wq = []
    for i in range(ND):
        t = wmat.tile([P, D], BF16, tag=f"w{i}", name=f"wq{i}")
        nc.gpsimd.dma_start(out=t, in_=d["Wq"][128 * i : 128 * (i + 1), :])
        wq.append(t)
    wo = []
    for i in range(ND):
        t = wop.tile([P, D], BF16, tag=f"wo{i}")
        nc.gpsimd.dma_start(out=t, in_=d["Wo"][128 * i : 128 * (i + 1), :])
        wo.append(t)

    # ================= main loop =================
    def emit_A(b, c):
        """x load + transpose + Q projection (unscaled bf16 qT)."""
        bc = c * B + b
        xin = []
        for m in range(4):
            t = xin_p.tile([P, D], BF16, tag=f"xin{m}", name=f"xin{m}")
            nc.gpsimd.dma_start(out=t, in_=d["x"][bc, 128 * m : 128 * (m + 1), :])
            xin.append(t)
        xt = []
        for i in range(ND):
            ps = pm2.tile([P, SL], BF16, tag="pm")
            sl = slice(128 * i, 128 * (i + 1))
            for m in range(4):
                nc.tensor.transpose(ps[:, 128 * m : 128 * (m + 1)], xin[m][:, sl], ident)
            t = xt_p.tile([P, SL], BF16, tag=f"xt{i}", name=f"xt{i}")
            nc.scalar.activation(t, ps, AF.Copy)
            xt.append(t)
        qt = []
        for j in range(ND):
            ps = pmm.tile([P, SL], F32, tag="ps")
            for i in range(ND):
                nc.tensor.matmul(
                    ps,
                    lhsT=wq[i][:, 128 * j : 128 * (j + 1)],
                    rhs=xt[i],
                    start=(i == 0),
                    stop=(i == ND - 1),
                )
            t = qt_p.tile([P, SL], BF16, tag=f"qt{j}", name=f"qt{j}")
            nc.vector.tensor_copy(t, ps)  # f32 -> bf16 cast on DVE
            qt.append(t)
        return qt

    def emit_B(b, c, qt, ppooled, denps):
        """scores -> exp -> AV -> den; pooled matmuls; returns ao tile."""
        bc = c * B + b
        rows = slice(32 * c, 32 * c + H)

        # pooled (mean-key) collector: rows 32c+h of the shared bank
        for j in range(ND):
            nc.tensor.matmul(
                ppooled[rows, :],
                lhsT=ksb[(bc, j)],
                rhs=qt[j],
                start=(j == 0),
                stop=(j == ND - 1),
                tile_position=(0, 32 * c),
            )

        # E1 scores, 4 heads packed per [128, 512] bank -> one exp each
        wb4 = []
        for p in range(4):
            psb = pm2.tile([P, SL], F32, tag="pm")
            for g in range(4):
                h = 4 * p + g
                j, hr = h // 2, 64 * (h % 2)
                nc.tensor.matmul(
                    psb[32 * g : 32 * (g + 1), :],
                    lhsT=kt[(bc, j)][hr : hr + 64, E0:E],
                    rhs=qt[j][hr : hr + 64, :],
                    start=(g == 0),
                    stop=(g == 3),
                    tile_position=(hr, 32 * g),
                )
            wt = wb_p.tile([P, SL], BF16, tag="wb4")
            nc.scalar.activation(wt, psb, AF.Exp, scale=SCALE)
            wb4.append(wt)
            # E1 part of den: one matmul per pack via block-diag one-hot
            nc.tensor.matmul(
                denps[rows, :], lhsT=ones4[p], rhs=wt, start=(p == 0), stop=False, tile_position=(0, 32 * c)
            )

        ao = ao_p.tile([P, ND * SL], BF16, tag=f"ao{c}", name=f"ao{c}")
        for h in range(H):
            j, hr = h // 2, 64 * (h % 2)
            g, p = h % 4, h // 4
            ps_a = psa.tile([P, SL], F32, tag="ps")
            nc.tensor.matmul(ps_a, lhsT=kt[(bc, j)][hr : hr + 64, 0:E0], rhs=qt[j][hr : hr + 64, :], start=True, stop=True)
            wa = w_p.tile([P, SL], BF16, tag="wa")
            nc.scalar.activation(wa, ps_a, AF.Exp, scale=SCALE)
            # E0 part of den for head h
            nc.tensor.matmul(
                denps[rows, :], lhsT=oneh[h], rhs=wa, start=False, stop=(h == H - 1), tile_position=(0, 32 * c)
            )
            # attention-value product (unnormalized)
            ps_av = pm2.tile([DH, SL], F32, tag="pm")
            nc.tensor.matmul(ps_av, lhsT=v0[bc][:, DH * h : DH * (h + 1)], rhs=wa, start=True, stop=False)
            nc.tensor.matmul(
                ps_av,
                lhsT=v14[bc][32 * g : 32 * g + E1, DH * h : DH * (h + 1)],
                rhs=wb4[p][32 * g : 32 * (g + 1), :],
                start=False,
                stop=True,
                tile_position=(32 * g, 0),
            )
            nc.vector.tensor_copy(ao[hr : hr + 64, SL * j : SL * (j + 1)], ps_av)
        return ao

    def emit_C_head(c, ppooled, denps, ep, rd):
        """per-component stats that can run early: exp(pooled), 1/den."""
        rows = slice(32 * c, 32 * c + H)
        nc.scalar.activation(ep[c], ppooled[rows, :], AF.Exp, scale=SCALE)
        with nc.allow_low_precision(reason="coef stats in bf16, ~0.4% rel on a 2e-2 budget"):
            nc.vector.reciprocal(rd[c], denps[rows, :])

    def emit_C_tail(b, ep, rd, coef_d):
        """softmax over components + fold in 1/den; spill coef to DRAM."""
        with nc.allow_low_precision(reason="coef stats in bf16, ~0.4% rel on a 2e-2 budget"):
            sc = stats.tile([H, SL], BF16, tag="sc")
            nc.vector.tensor_add(sc, ep[0], ep[1])
            nc.vector.tensor_add(sc, sc, ep[2])
            nc.vector.tensor_add(sc, sc, ep[3])
            rs = stats.tile([H, SL], BF16, tag="rs")
            nc.vector.reciprocal(rs, sc)
            for c in range(C):
                nc.vector.tensor_mul(ep[c], ep[c], rs)
                nc.vector.tensor_mul(ep[c], ep[c], rd[c])
                # scalar HWDGE queue: keeps coef traffic off the sync queue,
                # whose FIFO would stall these behind output stores
                nc.scalar.dma_start(out=coef_d[H * c : H * (c + 1), :], in_=ep[c])

    def emit_D(c, ao, coef_d):
        """ao *= coef, broadcast over dh via DRAM stride-0 partition reads."""
        for j in range(ND):
            cbt = cb_p.tile([P, SL], BF16, tag="cb")
            src2 = coef_d[c * H + 2 * j : c * H + 2 * j + 2, :]
            src2 = bass.AP(
                tensor=src2.tensor,
                offset=src2.offset,
                ap=[list(src2.ap[0]), [0, 64]] + [list(a) for a in src2.ap[1:]],
            )
            nc.scalar.dma_start(out=cbt, in_=src2)
            sl_ao = ao[:, SL * j : SL * (j + 1)]
            nc.vector.tensor_mul(sl_ao, sl_ao, cbt)

    def emit_E(b, c, ao):
        """out = aoT^T @ Wo + bo + x, streamed to DRAM."""
        bc = c * B + b
        for m in range(4):
            rows = slice(128 * m, 128 * (m + 1))
            for half in range(2):
                cols = slice(512 * half, 512 * (half + 1))
                ps = pmm.tile([P, 512], F32, tag="ps")
                nc.tensor.matmul(ps, lhsT=ones_row, rhs=bo_bf[:, cols], start=True, stop=False)
                for i in range(ND):
                    nc.tensor.matmul(
                        ps,
                        lhsT=ao[:, SL * i + 128 * m : SL * i + 128 * (m + 1)],
                        rhs=wo[i][:, cols],
                        start=False,
                        stop=(i == ND - 1),
                    )
                xr = xr_p.tile([P, 512], F32, tag="xr")
                nc.gpsimd.dma_start(out=xr, in_=d["x"][bc, rows, cols])
                nc.vector.tensor_add(xr, ps, xr)
                nc.sync.dma_start(out=d["out"][bc, rows, cols], in_=xr)

    # software pipeline: D/E of the previous (b, c) are emitted between A and B
    # of the current one; coef (C-tail) couples all four components of a b-iter.
    pend = None  # (b, [ao0..ao3], coef_d) awaiting D/E
    for b in range(B):
        ppooled = ppool.tile([P, SL], F32, tag="pp")
        denps = pden.tile([P, SL], F32, tag="pd")
        ep = [stats.tile([H, SL], BF16, tag=f"ep{c}", name=f"ep{c}") for c in range(C)]
        rd = [stats.tile([H, SL], BF16, tag=f"rd{c}", name=f"rd{c}") for c in range(C)]
        coef_d = dram.tile([C * H, SL], BF16, tag="coefd", name="coef_d", bufs=2)
        aos = []
        for c in range(C):
            qt = emit_A(b, c)
            if pend is not None:
                pb, paos, pcoef = pend
                emit_D(c, paos[c], pcoef)
                emit_E(pb, c, paos[c])
                if c == C - 1:
                    pend = None
            ao = emit_B(b, c, qt, ppooled, denps)
            aos.append(ao)
            emit_C_head(c, ppooled, denps, ep, rd)
        emit_C_tail(b, ep, rd, coef_d)
        pend = (b, aos, coef_d)
    # final flush: all coef-broadcast multiplies first, then the output
    # projections, so E(c) PE work overlaps D(c+1)'s DMA+DVE chain
    pb, paos, pcoef = pend
    for c in range(C):
        emit_D(c, paos[c], pcoef)
    for c in range(C):
        emit_E(pb, c, paos[c])


def build_program(s_loc=S_LOC, n_cores=N_CORES):
    nc = bacc.Bacc(trn_type="TRN2", target_bir_lowering=False, debug=False, num_devices=n_cores)
    d = {
        "x": nc.dram_tensor("x", [BC, s_loc, D], F32, kind="ExternalInput").ap(),
        "enc": nc.dram_tensor("enc", [BC, E, D], F32, kind="ExternalInput").ap(),
        "Wq": nc.dram_tensor("Wq", [D, D], F32, kind="ExternalInput").ap(),
        "Wk": nc.dram_tensor("Wk", [D, D], F32, kind="ExternalInput").ap(),
        "Wv": nc.dram_tensor("Wv", [D, D], F32, kind="ExternalInput").ap(),
        "Wo": nc.dram_tensor("Wo", [D, D], F32, kind="ExternalInput").ap(),
        "bo": nc.dram_tensor("bo", [1, D], F32, kind="ExternalInput").ap(),
        "out": nc.dram_tensor("out", [BC, s_loc, D], F32, kind="ExternalOutput").ap(),
    }
    with TileContext(nc, trace_sim=False) as tc, ExitStack() as ctx:
        build_body(ctx, tc, d, s_loc)
    nc.compile()
    return nc


def make_in_maps(hidden_states, encoder_hidden_states, Wq, Wk, Wv, Wo, bo, s_loc=S_LOC, n_cores=N_CORES):
    common = {
        "enc": np.ascontiguousarray(encoder_hidden_states, dtype=np.float32),
        "Wq": np.ascontiguousarray(Wq, dtype=np.float32),
        "Wk": np.ascontiguousarray(Wk, dtype=np.float32),
        "Wv": np.ascontiguousarray(Wv, dtype=np.float32),
        "Wo": np.ascontiguousarray(Wo, dtype=np.float32),
        "bo": np.ascontiguousarray(bo, dtype=np.float32).reshape(1, D),
    }
    return [
        {"x": np.ascontiguousarray(hidden_states[:, i * s_loc : (i + 1) * s_loc, :], dtype=np.float32), **common}
        for i in range(n_cores)
    ]


_NC = None


def kernel(hidden_states, encoder_hidden_states, Wq, Wk, Wv, Wo, bo):
    global _NC
    if _NC is None:
        _NC = build_program()
    in_maps = make_in_maps(hidden_states, encoder_hidden_states, Wq, Wk, Wv, Wo, bo)
    res = run_bass_kernel_spmd(_NC, in_maps, list(range(N_CORES))).results
    out = np.concatenate([res[i]["out"] for i in range(N_CORES)], axis=1)
    return np.ascontiguousarray(out, dtype=np.float32)


if __name__ == "__main__":
    rng = np.random.default_rng(0)
    ins = {
        "hidden_states": rng.standard_normal((BC, S, D), dtype=np.float32),
        "encoder_hidden_states": rng.standard_normal((BC, E, D), dtype=np.float32),
        "Wq": rng.standard_normal((D, D), dtype=np.float32) * 0.02,
        "Wk": rng.standard_normal((D, D), dtype=np.float32) * 0.02,
        "Wv": rng.standard_normal((D, D), dtype=np.float32) * 0.02,
        "Wo": rng.standard_normal((D, D), dtype=np.float32) * 0.02,
        "bo": np.zeros((D,), np.float32),
    }
    out = kernel(**ins)
    print("out", out.shape, out.dtype, float(np.abs(out).max()))


# revision 27
# speedup vs baseline: 1.0675x; 1.0675x over previous
"""Trainium2 Bass kernel for DecomposingAttnProcessor (pooled component softmax
cross-attention), sharded over 8 NeuronCores along the latent-token axis S.

Math (per batch-component bc = c*B + b):
    q = x @ Wq ; k = enc @ Wk ; v = enc @ Wv           (per-head, dh = 64)
    scores = (q k^T) * dh^-0.5                          [H, S, E]
    pooled = mean_E scores ; wp = softmax_c(pooled)
    w = softmax_E(scores) * wp
    out = (w v) @ Wo + bo + x

Sharding: each core owns a 512-row slice of S for ALL batch-components; the
component softmax couples only the c axis, which stays on-core.

v2 dataflow (512-row tiles, KV resident in SBUF):
    encoder phase: kT[(bc,j)] = Wk^T encT with fused ksum/E col (projected mean
      key for the pooled path), v0/v1 in natural [e, 64h+d] layout with the E1
      rows replicated to all four 32-row blocks (feeds row-tiled AV matmuls).
    per (b, c):  A: xT via PE transpose; qT = Wq^T xT (bf16, unscaled)
                 B: scoresT per head ([128,512] E0 + 4-head-packed E1 bank),
                    exp with scale=dh^-0.5 fused; AV; den via one-hot matmuls
                    accumulated into a [16,512] PSUM slice at rows 32c
                 C: coef = softmax_c(exp(scale*pooled)) / den  (bf16 stats)
                 D: coef broadcast over dh via DRAM bounce; ao *= coef
                 E: out = aoT^T @ Wo + bo + x
    D/E of (b,c) are emitted one component later so the PE stream stays dense
    while the coef chain resolves.
"""

import sys
from contextlib import ExitStack

sys.path.insert(0, "/opt/trn_rl_repo")

import numpy as np

import concourse.bass as bass  # noqa: E402
from concourse import bacc, mybir  # noqa: E402
from concourse.bass_utils import run_bass_kernel_spmd  # noqa: E402
from concourse.masks import make_identity  # noqa: E402
from concourse.tile import TileContext  # noqa: E402

# Problem dims (hardcoded per spec)
BC, S, D, E, H, C = 8, 4096, 1024, 160, 16, 4
B = BC // C  # 2
DH = D // H  # 64
SCALE = DH**-0.5  # 0.125
N_CORES = 8
S_LOC = S // N_CORES  # 512 rows of S per core
E0, E1 = 128, E - 128  # encoder-token chunks (128 + 32)
ND = D // 128  # 8 chunks of the hidden dim

F32 = mybir.dt.float32
BF16 = mybir.dt.bfloat16
AF = mybir.ActivationFunctionType


def build_body(ctx, tc, d, s_loc):
    nc = tc.nc
    P = 128
    SL = s_loc  # 512

    pools = {}

    def pool(name, bufs, space="SBUF"):
        if name not in pools:
            pools[name] = ctx.enter_context(tc.tile_pool(name=name, bufs=bufs, space=space))
        return pools[name]

    const = pool("const", 1)
    wmat = pool("wmat", 1)  # Wk|Wv bf16 pairs, later reused for Wq
    wop = pool("wo", 1)
    ktp = pool("kt", 1)  # resident kT tiles, all 8 bc
    vp = pool("v", 1)  # resident v0 / v1_4 tiles, all 8 bc
    ksbp = pool("ksb", 1)  # resident block-diag ksum tiles
    xin_p = pool("xin", 1)
    xt_p = pool("xt", 1)
    qt_p = pool("qt", 2)
    w_p = pool("w", 3)  # exp(scores) E0 per-head tiles
    wb_p = pool("wb4", 4)  # exp(scores) E1 4-head-packed tiles
    ao_p = pool("ao", 1)
    stats = pool("stats", 1)
    cb_p = pool("cb", 4)
    xr_p = pool("xr", 2)
    dram = pool("dram", 1, space="DRAM")

    # PSUM pools: 8 banks total
    pmm = pool("pmm", 2, space="PSUM")  # qproj / vproj / O-proj [128,512] f32
    psa = pool("psa", 2, space="PSUM")  # E0 scores [128,512] f32
    pm2 = pool("pm2", 2, space="PSUM")  # transposes / E1-pack / AV
    ppool = pool("ppool", 1, space="PSUM")  # pooled collector, rows 32c+h
    pden = pool("pden", 1, space="PSUM")  # denominator collector, rows 32c+h

    # ---- constants ----
    ident = const.tile([P, P], BF16, tag="ident")
    make_identity(nc, ident)
    ones_row = const.tile([1, P], BF16, tag="ones_row")
    nc.vector.memset(ones_row, 1.0)
    bo_bf = const.tile([1, D], BF16, tag="bo_bf")
    nc.gpsimd.dma_start(out=bo_bf, in_=d["bo"])  # f32 -> bf16 cast DMA

    # one-hot selector columns for the den matmuls:
    #   oneh[h]: [128, 16] with col h = 1 (E0 full-column sum for head h)
    #   ones4[p]: [128, 16] with rows 32g:32g+32 of col 4p+g = 1 (E1 pack p)
    oneh = []
    for h in range(H):
        t = const.tile([P, H], BF16, tag=f"oneh{h}", name=f"oneh{h}")
        nc.vector.memset(t, 0.0)
        nc.vector.memset(t[:, h : h + 1], 1.0)
        oneh.append(t)
    ones4 = []
    for p in range(4):
        t = const.tile([P, H], BF16, tag=f"ones4_{p}", name=f"ones4_{p}")
        nc.vector.memset(t, 0.0)
        for g in range(4):
            nc.vector.memset(t[32 * g : 32 * (g + 1), 4 * p + g : 4 * p + g + 1], 1.0)
        ones4.append(t)

    # ---- load Wk/Wv as bf16 pairs (Wk halves first so the kT chain starts
    # as soon as possible; Wo/bo are loaded after the encoder is emitted) ----
    wkv = []
    for i in range(ND):
        t = wmat.tile([P, 2 * D], BF16, tag=f"w{i}", name=f"wkv{i}")
        nc.gpsimd.dma_start(out=t[:, 0:D], in_=d["Wk"][128 * i : 128 * (i + 1), :])
        wkv.append(t)
    for i in range(ND):
        nc.gpsimd.dma_start(out=wkv[i][:, D : 2 * D], in_=d["Wv"][128 * i : 128 * (i + 1), :])

    # ---- resident KV tiles ----
    kt = {}  # (bc, j) -> [128, E+1] bf16 (col E = projected ksum/E)
    v0 = {}  # bc -> [128, 1024] bf16, rows = E0 tokens, cols = 64h + d
    v14 = {}  # bc -> [128, 1024] bf16, E1 tokens replicated at rows 32g
    ksb = {}  # (bc, j) -> [128, 16] view into ksb_all
    for bc in range(BC):
        for j in range(ND):
            kt[(bc, j)] = ktp.tile([P, E + 1], BF16, tag=f"kt{bc}_{j}", name=f"kt{bc}_{j}")
        v0[bc] = vp.tile([P, H * DH], BF16, tag=f"v0_{bc}", name=f"v0_{bc}")
        v14[bc] = vp.tile([P, H * DH], BF16, tag=f"v14_{bc}", name=f"v14_{bc}")

    # ---- encoder phase: per bc, compute kT (+ksum/E col) and v ----
    with tc.tile_pool(name="enc", bufs=2) as encp:
        for bc in range(BC):
            et0 = encp.tile([P, D], BF16, tag="et0")
            et1 = encp.tile([E1, D], BF16, tag="et1")
            nc.gpsimd.dma_start(out=et0, in_=d["enc"][bc, 0:E0, :])
            nc.gpsimd.dma_start(out=et1, in_=d["enc"][bc, E0:E, :])

            # encT, all 8 chunks in one [128, 8*(E+1)] tile; col 160 of each
            # chunk = esum/E (batched reduce + strided scatter-mul)
            enct = encp.tile([P, ND * (E + 1)], BF16, tag="enct", bufs=1)
            for i in range(ND):
                ps = pm2.tile([P, E], BF16, tag="pm")
                sl = slice(128 * i, 128 * (i + 1))
                nc.tensor.transpose(ps[:, 0:E0], et0[:, sl], ident)
                nc.tensor.transpose(ps[:, E0:E], et1[:, sl], ident[0:E1, 0:E1])
                nc.scalar.activation(enct[:, (E + 1) * i : (E + 1) * i + E], ps, AF.Copy)
            esum = encp.tile([P, ND], F32, tag="esum", bufs=1)
            e3 = enct.rearrange("p (i e) -> p i e", e=E + 1)
            nc.vector.tensor_reduce(esum, e3[:, :, 0:E], axis=mybir.AxisListType.X, op=mybir.AluOpType.add)
            nc.scalar.mul(e3[:, :, E : E + 1], esum, 1.0 / E)

            # kT projection: [dout-chunk, E+1] (col E = projected ksum/E)
            ksb_all = ksbp.tile([P, ND * H], BF16, tag=f"ksb{bc}", name=f"ksb{bc}")
            nc.vector.memset(ksb_all, 0.0)
            for j in range(ND):
                ps = psa.tile([P, E + 1], F32, tag="ps")
                for i in range(ND):
                    nc.tensor.matmul(
                        ps,
                        lhsT=wkv[i][:, 128 * j : 128 * (j + 1)],
                        rhs=enct[:, (E + 1) * i : (E + 1) * (i + 1)],
                        start=(i == 0),
                        stop=(i == ND - 1),
                    )
                t = kt[(bc, j)]
                nc.scalar.activation(t, ps, AF.Copy)
                ksb[(bc, j)] = ksb_all[:, H * j : H * (j + 1)]
                nc.vector.tensor_copy(ksb_all[0:64, H * j + 2 * j : H * j + 2 * j + 1], t[0:64, E : E + 1])
                nc.vector.tensor_copy(ksb_all[64:128, H * j + 2 * j + 1 : H * j + 2 * j + 2], t[64:128, E : E + 1])

            # v projection, natural [token, 64h+d] layout
            for half in range(2):
                cs = slice(D + 512 * half, D + 512 * (half + 1))
                ps = pmm.tile([P, 512], F32, tag="ps")
                for i in range(ND):
                    nc.tensor.matmul(
                        ps,
                        lhsT=enct[:, (E + 1) * i : (E + 1) * i + E0],
                        rhs=wkv[i][:, cs],
                        start=(i == 0),
                        stop=(i == ND - 1),
                    )
                nc.scalar.activation(v0[bc][:, 512 * half : 512 * (half + 1)], ps, AF.Copy)
                ps1 = psa.tile([E1, 512], F32, tag="ps")
                for i in range(ND):
                    nc.tensor.matmul(
                        ps1,
                        lhsT=enct[:, (E + 1) * i + E0 : (E + 1) * i + E],
                        rhs=wkv[i][:, cs],
                        start=(i == 0),
                        stop=(i == ND - 1),
                    )
                nc.scalar.activation(v14[bc][0:E1, 512 * half : 512 * (half + 1)], ps1, AF.Copy)
            for g in range(1, 4):
                nc.vector.tensor_copy(v14[bc][32 * g : 32 * g + E1, :], v14[bc][0:E1, :])

    # ---- load Wq (bf16) into the Wk/Wv slots; Wo/bo now too ----
    wq = []
    for i in range(ND):
        t = wmat.tile([P, D], BF16, tag=f"w{i}", name=f"wq{i}")
        nc.gpsimd.dma_start(out=t, in_=d["Wq"][128 * i : 128 * (i + 1), :])
        wq.append(t)
    wo = []
    for i in range(ND):
        t = wop.tile([P, D], BF16, tag=f"wo{i}")
        nc.gpsimd.dma_start(out=t, in_=d["Wo"][128 * i : 128 * (i + 1), :])
        wo.append(t)

    # ================= main loop =================
    def emit_A(b, c):
        """x load + transpose + Q projection (unscaled bf16 qT)."""
        bc = c * B + b
        xin = []
        for m in range(4):
            t = xin_p.tile([P, D], BF16, tag=f"xin{m}", name=f"xin{m}")
            nc.gpsimd.dma_start(out=t, in_=d["x"][bc, 128 * m : 128 * (m + 1), :])
            xin.append(t)
        xt = []
        for i in range(ND):
            ps = pm2.tile([P, SL], BF16, tag="pm")
            sl = slice(128 * i, 128 * (i + 1))
            for m in range(4):
                nc.tensor.transpose(ps[:, 128 * m : 128 * (m + 1)], xin[m][:, sl], ident)
            t = xt_p.tile([P, SL], BF16, tag=f"xt{i}", name=f"xt{i}")
            # alternate engines: the PE transposes outrun a single engine's
            # eviction copies ~3x, which would stall the pm2 pool rotation
            if i % 2 == 0:
                nc.scalar.activation(t, ps, AF.Copy)
            else:
                nc.vector.tensor_copy(t, ps)
            xt.append(t)
        qt = []
        for j in range(ND):
            ps = pmm.tile([P, SL], F32, tag="ps")
            for i in range(ND):
                nc.tensor.matmul(
                    ps,
                    lhsT=wq[i][:, 128 * j : 128 * (j + 1)],
                    rhs=xt[i],
                    start=(i == 0),
                    stop=(i == ND - 1),
                )
            t = qt_p.tile([P, SL], BF16, tag=f"qt{j}", name=f"qt{j}")
            nc.vector.tensor_copy(t, ps)  # f32 -> bf16 cast on DVE
            qt.append(t)
        return qt

    def emit_B(b, c, qt, ppooled, denps):
        """scores -> exp -> AV -> den, software-pipelined one head ahead so the
        in-order PE queue never sits directly behind an exp dependency."""
        bc = c * B + b
        rows = slice(32 * c, 32 * c + H)

        # E1 scores, 4 heads packed per [128, 512] bank -> one exp each
        wb4 = []
        for p in range(4):
            psb = pm2.tile([P, SL], F32, tag="pm")
            for g in range(4):
                h = 4 * p + g
                j, hr = h // 2, 64 * (h % 2)
                nc.tensor.matmul(
                    psb[32 * g : 32 * (g + 1), :],
                    lhsT=kt[(bc, j)][hr : hr + 64, E0:E],
                    rhs=qt[j][hr : hr + 64, :],
                    start=(g == 0),
                    stop=(g == 3),
                    tile_position=(hr, 32 * g),
                )
            wt = wb_p.tile([P, SL], BF16, tag="wb4")
            nc.scalar.activation(wt, psb, AF.Exp, scale=SCALE)
            wb4.append(wt)

        # pooled (mean-key) collector: PE filler while the E1 exps resolve
        for j in range(ND):
            nc.tensor.matmul(
                ppooled[rows, :],
                lhsT=ksb[(bc, j)],
                rhs=qt[j],
                start=(j == 0),
                stop=(j == ND - 1),
                tile_position=(0, 32 * c),
            )

        def emit_score(h):
            j, hr = h // 2, 64 * (h % 2)
            ps_a = psa.tile([P, SL], F32, tag="ps")
            nc.tensor.matmul(ps_a, lhsT=kt[(bc, j)][hr : hr + 64, 0:E0], rhs=qt[j][hr : hr + 64, :], start=True, stop=True)
            wa = w_p.tile([P, SL], BF16, tag="wa")
            nc.scalar.activation(wa, ps_a, AF.Exp, scale=SCALE)
            return wa

        ao = ao_p.tile([P, ND * SL], BF16, tag=f"ao{c}", name=f"ao{c}")
        wa_pend = emit_score(0)
        # E1 den matmuls sit here: by now all four E1 exps are long done
        for p in range(4):
            nc.tensor.matmul(
                denps[rows, :], lhsT=ones4[p], rhs=wb4[p], start=(p == 0), stop=False, tile_position=(0, 32 * c)
            )
        for h in range(H):
            wa = wa_pend
            wa_pend = emit_score(h + 1) if h + 1 < H else None
            j, hr = h // 2, 64 * (h % 2)
            g, p = h % 4, h // 4
            # E0 part of den for head h
            nc.tensor.matmul(
                denps[rows, :], lhsT=oneh[h], rhs=wa, start=False, stop=(h == H - 1), tile_position=(0, 32 * c)
            )
            # attention-value product (unnormalized)
            ps_av = pm2.tile([DH, SL], F32, tag="pm")
            nc.tensor.matmul(ps_av, lhsT=v0[bc][:, DH * h : DH * (h + 1)], rhs=wa, start=True, stop=False)
            nc.tensor.matmul(
                ps_av,
                lhsT=v14[bc][32 * g : 32 * g + E1, DH * h : DH * (h + 1)],
                rhs=wb4[p][32 * g : 32 * (g + 1), :],
                start=False,
                stop=True,
                tile_position=(32 * g, 0),
            )
            nc.vector.tensor_copy(ao[hr : hr + 64, SL * j : SL * (j + 1)], ps_av)
        return ao

    def emit_C_head(c, ppooled, denps, ep, rd):
        """per-component stats that can run early: exp(pooled), 1/den."""
        rows = slice(32 * c, 32 * c + H)
        nc.scalar.activation(ep[c], ppooled[rows, :], AF.Exp, scale=SCALE)
        with nc.allow_low_precision(reason="coef stats in bf16, ~0.4% rel on a 2e-2 budget"):
            nc.vector.reciprocal(rd[c], denps[rows, :])

    def emit_C_tail(b, ep, rd, coef_d):
        """softmax over components + fold in 1/den; spill coef to DRAM."""
        with nc.allow_low_precision(reason="coef stats in bf16, ~0.4% rel on a 2e-2 budget"):
            sc = stats.tile([H, SL], BF16, tag="sc")
            nc.vector.tensor_add(sc, ep[0], ep[1])
            nc.vector.tensor_add(sc, sc, ep[2])
            nc.vector.tensor_add(sc, sc, ep[3])
            rs = stats.tile([H, SL], BF16, tag="rs")
            nc.vector.reciprocal(rs, sc)
            for c in range(C):
                nc.vector.tensor_mul(ep[c], ep[c], rs)
                nc.vector.tensor_mul(ep[c], ep[c], rd[c])
                # scalar HWDGE queue: keeps coef traffic off the sync queue,
                # whose FIFO would stall these behind output stores
                nc.scalar.dma_start(out=coef_d[H * c : H * (c + 1), :], in_=ep[c])

    def emit_D(c, ao, coef_d):
        """ao *= coef, broadcast over dh via DRAM stride-0 partition reads."""
        for j in range(ND):
            cbt = cb_p.tile([P, SL], BF16, tag="cb")
            src2 = coef_d[c * H + 2 * j : c * H + 2 * j + 2, :]
            src2 = bass.AP(
                tensor=src2.tensor,
                offset=src2.offset,
                ap=[list(src2.ap[0]), [0, 64]] + [list(a) for a in src2.ap[1:]],
            )
            nc.scalar.dma_start(out=cbt, in_=src2)
            sl_ao = ao[:, SL * j : SL * (j + 1)]
            nc.vector.tensor_mul(sl_ao, sl_ao, cbt)

    def emit_E(b, c, ao):
        """out = aoT^T @ Wo + bo + x, streamed to DRAM."""
        bc = c * B + b
        for m in range(4):
            rows = slice(128 * m, 128 * (m + 1))
            for half in range(2):
                cols = slice(512 * half, 512 * (half + 1))
                ps = pmm.tile([P, 512], F32, tag="ps")
                nc.tensor.matmul(ps, lhsT=ones_row, rhs=bo_bf[:, cols], start=True, stop=False)
                for i in range(ND):
                    nc.tensor.matmul(
                        ps,
                        lhsT=ao[:, SL * i + 128 * m : SL * i + 128 * (m + 1)],
                        rhs=wo[i][:, cols],
                        start=False,
                        stop=(i == ND - 1),
                    )
                xr = xr_p.tile([P, 512], F32, tag="xr")
                nc.gpsimd.dma_start(out=xr, in_=d["x"][bc, rows, cols])
                nc.vector.tensor_add(xr, ps, xr)
                nc.sync.dma_start(out=d["out"][bc, rows, cols], in_=xr)

    # software pipeline: D/E of the previous (b, c) are emitted between A and B
    # of the current one; coef (C-tail) couples all four components of a b-iter.
    pend = None  # (b, [ao0..ao3], coef_d) awaiting D/E
    for b in range(B):
        ppooled = ppool.tile([P, SL], F32, tag="pp")
        denps = pden.tile([P, SL], F32, tag="pd")
        ep = [stats.tile([H, SL], BF16, tag=f"ep{c}", name=f"ep{c}") for c in range(C)]
        rd = [stats.tile([H, SL], BF16, tag=f"rd{c}", name=f"rd{c}") for c in range(C)]
        coef_d = dram.tile([C * H, SL], BF16, tag="coefd", name="coef_d", bufs=2)
        aos = []
        for c in range(C):
            qt = emit_A(b, c)
            if pend is not None:
                pb, paos, pcoef = pend
                emit_D(c, paos[c], pcoef)
                emit_E(pb, c, paos[c])
                if c == C - 1:
                    pend = None
            ao = emit_B(b, c, qt, ppooled, denps)
            aos.append(ao)
            emit_C_head(c, ppooled, denps, ep, rd)
        emit_C_tail(b, ep, rd, coef_d)
        pend = (b, aos, coef_d)
    # final flush: all coef-broadcast multiplies first, then the output
    # projections, so E(c) PE work overlaps D(c+1)'s DMA+DVE chain
    pb, paos, pcoef = pend
    for c in range(C):
        emit_D(c, paos[c], pcoef)
    for c in range(C):
        emit_E(pb, c, paos[c])


def build_program(s_loc=S_LOC, n_cores=N_CORES):
    nc = bacc.Bacc(trn_type="TRN2", target_bir_lowering=False, debug=False, num_devices=n_cores)
    d = {
        "x": nc.dram_tensor("x", [BC, s_loc, D], F32, kind="ExternalInput").ap(),
        "enc": nc.dram_tensor("enc", [BC, E, D], F32, kind="ExternalInput").ap(),
        "Wq": nc.dram_tensor("Wq", [D, D], F32, kind="ExternalInput").ap(),
        "Wk": nc.dram_tensor("Wk", [D, D], F32, kind="ExternalInput").ap(),
        "Wv": nc.dram_tensor("Wv", [D, D], F32, kind="ExternalInput").ap(),
        "Wo": nc.dram_tensor("Wo", [D, D], F32, kind="ExternalInput").ap(),
        "bo": nc.dram_tensor("bo", [1, D], F32, kind="ExternalInput").ap(),
        "out": nc.dram_tensor("out", [BC, s_loc, D], F32, kind="ExternalOutput").ap(),
    }
    with TileContext(nc, trace_sim=False) as tc, ExitStack() as ctx:
        build_body(ctx, tc, d, s_loc)
    nc.compile()
    return nc


def make_in_maps(hidden_states, encoder_hidden_states, Wq, Wk, Wv, Wo, bo, s_loc=S_LOC, n_cores=N_CORES):
    common = {
        "enc": np.ascontiguousarray(encoder_hidden_states, dtype=np.float32),
        "Wq": np.ascontiguousarray(Wq, dtype=np.float32),
        "Wk": np.ascontiguousarray(Wk, dtype=np.float32),
        "Wv": np.ascontiguousarray(Wv, dtype=np.float32),
        "Wo": np.ascontiguousarray(Wo, dtype=np.float32),
        "bo": np.ascontiguousarray(bo, dtype=np.float32).reshape(1, D),
    }
    return [
        {"x": np.ascontiguousarray(hidden_states[:, i * s_loc : (i + 1) * s_loc, :], dtype=np.float32), **common}
        for i in range(n_cores)
    ]


_NC = None


def kernel(hidden_states, encoder_hidden_states, Wq, Wk, Wv, Wo, bo):
    global _NC
    if _NC is None:
        _NC = build_program()
    in_maps = make_in_maps(hidden_states, encoder_hidden_states, Wq, Wk, Wv, Wo, bo)
    res = run_bass_kernel_spmd(_NC, in_maps, list(range(N_CORES))).results
    out = np.concatenate([res[i]["out"] for i in range(N_CORES)], axis=1)
    return np.ascontiguousarray(out, dtype=np.float32)


if __name__ == "__main__":
    rng = np.random.default_rng(0)
    ins = {
        "hidden_states": rng.standard_normal((BC, S, D), dtype=np.float32),
        "encoder_hidden_states": rng.standard_normal((BC, E, D), dtype=np.float32),
        "Wq": rng.standard_normal((D, D), dtype=np.float32) * 0.02,
        "Wk": rng.standard_normal((D, D), dtype=np.float32) * 0.02,
        "Wv": rng.standard_normal((D, D), dtype=np.float32) * 0.02,
        "Wo": rng.standard_normal((D, D), dtype=np.float32) * 0.02,
        "bo": np.zeros((D,), np.float32),
    }
    out = kernel(**ins)
    print("out", out.shape, out.dtype, float(np.abs(out).max()))
